# revision 42
# baseline (speedup 1.0000x reference)
"""Distributed multi-head attention for 8 TRN2 NeuronCores.

Problem: x[2,2048,1024] -> QKV proj (w_qkv[3072,1024]) -> 16-head SDPA ->
out proj (w_proj[1024,1024] + b_proj) -> [2,2048,1024].

Sharding: 2 heads per core (head-parallel over 8 cores; both batches on
every core). The kernel is one long software pipeline balancing the two
bottleneck engines: scalar-engine EXP (16.8M softmax elements/core ~=
139 us) and the PE (~185 us of matmuls at the observed ~2.0 GHz clock):

  head:  progressive (wT, x) kt-slice DMAs, then QKV for batch-0 chunk 0.
  body:  64 slot-pairs (2 batches x 4 qchunks x 8 key-tile pairs). Each
         pair emits S^T x4 (row-tiled K=64 matmuls, bunched to halve PE
         tile-mode-switch drains) -> EXP x2 -> fillers -> AV x4, where AV
         accumulates [V_h|1]^T P_h^T in PSUM (row 64 = softmax
         denominator).  Fillers occupy the PE during the EXP dependency
         window: remaining batch-0 QKV chunks (pairs 0-5), batch-1 QKV
         (8-29), out-projection for completed collectives (38-39, 52-53).
  norm:  per (qchunk, head): denominator copy, reciprocal_approx_fast,
         gpsimd partition-broadcast, multiply; straight out of PSUM for
         the final qchunk to shorten the chain gating the last exchange.
  A2A:   4 AllToAll collectives (one per half-batch, 0.25MB/core), fired
         as each half-batch is normalized; ~25us each on the Comms engine
         (latency-bound, count matters more than bytes), all but the last
         hidden under later attention.
  proj:  out rows = attnT.T @ w_proj.T + b_proj per 128-token strip;
         strips 0/1 run as fillers, strips 2/3 in the tail where their
         lhs wait overlaps the final collective.

Token ownership: core c owns tokens [1024*hb + 128*c, +128) of each batch
half hb; out_ext row block k=2*b+hb holds that strip. Host reassembles.

Measured: ~278-288us HW exec (baseline 312-340us), rel err 5.4e-3.
"""
import sys, os, types
import numpy as np

if "/opt/trn_rl_repo" not in sys.path and os.path.isdir("/opt/trn_rl_repo"):
    sys.path.append("/opt/trn_rl_repo")

import concourse.bass as bass
import concourse.mybir as mybir
import concourse.tile as tile
from concourse import bacc
from concourse.bass_utils import run_bass_kernel_spmd

F32 = mybir.dt.float32
BF16 = mybir.dt.bfloat16
EXP = mybir.ActivationFunctionType.Exp
MULT = mybir.AluOpType.mult
ADD = mybir.AluOpType.add

NCORES = 8
B, N, C, H, D = 2, 2048, 1024, 16, 64
NT = B * N          # 4096 flat tokens
KT = C // 128       # 8 contraction tiles of 128
QC = 512            # query-chunk width (one PSUM bank of f32)
NMT = N // 128      # 16 key tiles per batch
SCALE = 1.0 / 8.0   # 1/sqrt(D)
NCH = NT // QC      # 8 512-token QKV chunks (4 per batch)

TRACE = False       # test harness sets True to capture exec_time_ns
LAST_EXEC_NS = None

_NC = None


def _install_ntff_hook():
    if "antenv.axon_hooks" in sys.modules:
        return
    try:
        import antenv
        from trn_agent_boot.trn_boot import _ntff_profile_via_ctypes
        mod = types.ModuleType("antenv.axon_hooks")
        _hook = [None]
        mod.set_axon_ntff_profile_hook = lambda h: _hook.__setitem__(0, h)
        mod.get_axon_ntff_profile_hook = lambda: _hook[0]
        sys.modules["antenv.axon_hooks"] = mod
        antenv.axon_hooks = mod
        mod.set_axon_ntff_profile_hook(
            _ntff_profile_via_ctypes("/opt/axon/libaxon_pjrt.so"))
    except Exception:
        pass


def _build():
    nc = bacc.Bacc("TRN2", target_bir_lowering=False, debug=False,
                   num_devices=NCORES)
    xT_ext = nc.dram_tensor("xT", [C, NT], BF16, kind="ExternalInput").ap()
    wT_ext = nc.dram_tensor("wT", [C, 384], BF16, kind="ExternalInput").ap()
    wpT_ext = nc.dram_tensor("wpT", [C, C], BF16, kind="ExternalInput").ap()
    bias_ext = nc.dram_tensor("bias", [1, C], F32, kind="ExternalInput").ap()
    idn_ext = nc.dram_tensor("idn", [128, 128], BF16, kind="ExternalInput").ap()
    out_ext = nc.dram_tensor("out", [NT // NCORES, C], F32,
                             kind="ExternalOutput").ap()
    # one exchange per half-batch (128-token strips/core); each AllToAll
    # costs ~25us on the Comms engine regardless of payload, so fewer,
    # overlapped collectives beat finer splits
    a2a_cols = {0: 128, 1: 128, 2: 128, 3: 128}
    a2a_in = {k: nc.dram_tensor(f"a2a_in{k}", [NCORES * 128, w], BF16).ap()
              for k, w in a2a_cols.items()}
    a2a_out = {k: nc.dram_tensor(f"a2a_out{k}", [NCORES * 128, w], BF16).ap()
               for k, w in a2a_cols.items()}

    xT_v = xT_ext.rearrange("(kt p) n -> p kt n", p=128)
    wT_v = wT_ext.rearrange("(kt p) f -> p kt f", p=128)
    wpT_v = wpT_ext.rearrange("(kt p) f -> p kt f", p=128)

    with tile.TileContext(nc) as tc:
        with (
            tc.tile_pool(name="const", bufs=1) as cpool,
            tc.tile_pool(name="resid", bufs=1) as rpool,
            tc.tile_pool(name="xchunk", bufs=1) as xpool,
            tc.tile_pool(name="vtmp", bufs=2) as vpool,
            tc.tile_pool(name="pexp", bufs=4) as ppool,
            tc.tile_pool(name="stg", bufs=2) as stpool,
            tc.tile_pool(name="denp", bufs=2) as dpool,
            tc.tile_pool(name="rbp", bufs=4) as rbpool,
            tc.tile_pool(name="onrm", bufs=4) as onpool,
            tc.tile_pool(name="plhs", bufs=2) as lpool,
            tc.tile_pool(name="pout", bufs=2) as outpool,
            tc.tile_pool(name="ps", bufs=1, space="PSUM") as pspool,
        ):
            # ---- constants + input DMAs (few large issues; DIRECT2D costs
            # ~0.6us of Sync-queue time per dma_start) ----
            wT_sb = cpool.tile([128, KT, 384], BF16)
            x_t = []
            for i in range(NT // 1024):
                xt = xpool.tile([128, KT, 1024], BF16, tag=f"x{i}",
                                name=f"x_{i}")
                x_t.append(xt)
            # first QKV group consumes (wT, x0) kt-slices in order: land
            # them progressively so the cold-start matmuls stream behind DMA
            for kt0 in range(0, KT, 2):
                nc.sync.dma_start(wT_sb[:, kt0:kt0 + 2, :],
                                  wT_v[:, kt0:kt0 + 2, :])
                nc.sync.dma_start(x_t[0][:, kt0:kt0 + 2, :],
                                  xT_v[:, kt0:kt0 + 2, 0:1024])
            idn = cpool.tile([128, 128], BF16)
            nc.sync.dma_start(idn[:], idn_ext[:])
            bias_sb = cpool.tile([1, C], F32)
            nc.sync.dma_start(bias_sb[:], bias_ext[:])
            bias_bc = cpool.tile([128, C], F32)
            nc.gpsimd.partition_broadcast(bias_bc[:], bias_sb[:])
            for i in range(1, 4):
                nc.sync.dma_start(x_t[i][:],
                                  xT_v[:, :, i * 1024:(i + 1) * 1024])

            qT_sb = rpool.tile([128, NT], BF16)
            kT_sb = rpool.tile([128, NT], BF16)
            v_sb = rpool.tile([128, NT // 128, 130], BF16)
            nc.gpsimd.memset(v_sb[:, :, 64], 1.0)
            nc.gpsimd.memset(v_sb[:, :, 129], 1.0)
            wp_sb = rpool.tile([128, KT, C], BF16)
            nc.sync.dma_start(wp_sb[:], wpT_v[:])

            vt_store = {}
            lhs_store = {}

            # ---- filler closures ----
            def qkv_unit(ch, ft, c0=0, c1=QC):
                """One QKV matmul group: tokens [c0,c1) of 512-token chunk
                ch x one feature block (ft 0=q, 1=k, 2=v) accumulated over
                KT, then evacuated."""
                def emit():
                    ti, hw = ch // 2, ch % 2
                    ncol = ch * QC + c0
                    w = c1 - c0
                    ps = pspool.tile([128, w], F32, tag="s", bufs=2,
                                     name=f"qkvps_{ch}_{ft}_{c0}")
                    for kt in range(KT):
                        nc.tensor.matmul(
                            ps[:],
                            wT_sb[:, kt, ft * 128:(ft + 1) * 128],
                            x_t[ti][:, kt, hw * QC + c0:hw * QC + c1],
                            start=(kt == 0), stop=(kt == KT - 1))
                    if ft == 0:
                        nc.vector.tensor_copy(out=qT_sb[:, ncol:ncol + w],
                                              in_=ps[:])
                    elif ft == 1:
                        nc.vector.tensor_copy(out=kT_sb[:, ncol:ncol + w],
                                              in_=ps[:])
                    else:
                        vt = vpool.tile([128, QC], BF16, tag="vt",
                                        name=f"vt_{ch}")
                        nc.vector.tensor_copy(out=vt[:], in_=ps[:])
                        vt_store[ch] = vt
                return emit

            def tr_unit(ch, t):
                """PE-transpose one 128-token tile of V into natural layout."""
                def emit():
                    mtg = ch * 4 + t
                    trp = pspool.tile([128, 128], BF16, tag="tr", bufs=2,
                                      name=f"tr_{mtg}")
                    nc.tensor.transpose(trp[:], vt_store[ch][:, t * 128:(t + 1) * 128],
                                        idn[:])
                    nc.vector.tensor_copy(out=v_sb[:, mtg, 0:64],
                                          in_=trp[:, 0:64])
                    nc.vector.tensor_copy(out=v_sb[:, mtg, 65:129],
                                          in_=trp[:, 64:128])
                return emit

            out_row = {0: 0, 1: 128, 2: 256, 3: 384}

            def proj_unit(k, half):
                """Out-projection for strip k, 512 output cols."""
                def emit():
                    lhs = lhs_store[k]
                    w = a2a_cols[k]
                    pp = pspool.tile([w, QC], F32, tag="s", bufs=2,
                                     name=f"pp_{k}_{half}")
                    for j in range(KT):
                        nc.tensor.matmul(
                            pp[:], lhs[:, j, :],
                            wp_sb[:, j, half * QC:(half + 1) * QC],
                            start=(j == 0), stop=(j == KT - 1))
                    ot = outpool.tile([w, QC], F32, tag="ot",
                                      name=f"ot_{k}_{half}")
                    nc.vector.tensor_tensor(ot[:], pp[:],
                                            bias_bc[0:w, half * QC:(half + 1) * QC],
                                            ADD)
                    nc.sync.dma_start(
                        out_ext[out_row[k]:out_row[k] + w,
                                half * QC:(half + 1) * QC], ot[:])
                return emit

            def chunk_units(ch):
                # k then q first: the first score matmul of a qchunk only
                # needs kT + qT, so exp starts before v is transposed
                return [qkv_unit(ch, 1), qkv_unit(ch, 0), qkv_unit(ch, 2),
                        tr_unit(ch, 0), tr_unit(ch, 1),
                        tr_unit(ch, 2), tr_unit(ch, 3)]

            def unit_norm(k, qloc, h, o_ps_h, fast=False):
                """Evacuate one finished (qchunk, head) PSUM accumulator,
                normalize by its softmax denominator, stream into a2a_in.
                fast=True (final qchunk) works straight out of PSUM to
                shorten the chain gating the last collective."""
                uid = f"{k}_{qloc}_{h}"
                dn = dpool.tile([1, QC], F32, tag="den", bufs=4,
                                name=f"den_{uid}")
                nc.vector.tensor_copy(out=dn[:], in_=o_ps_h[64:65, :])
                if fast:
                    src = o_ps_h[0:64, :]
                else:
                    st = stpool.tile([64, QC], F32, tag="st", bufs=4,
                                     name=f"st_{uid}")
                    nc.vector.tensor_copy(out=st[:], in_=o_ps_h[0:64, :])
                    src = st[:]
                rcp = dpool.tile([1, QC], F32, tag="rcp", bufs=4,
                                 name=f"rcp_{uid}")
                nc.vector.reciprocal_approx_fast(rcp[:], dn[:])
                rb = rbpool.tile([64, QC], F32, tag="rb", name=f"rb_{uid}")
                nc.gpsimd.partition_broadcast(rb[:], rcp[:])
                on = onpool.tile([64, QC], BF16, tag="on", name=f"on_{uid}")
                nc.vector.tensor_tensor(on[:], src, rb[:], MULT)
                if isinstance(k, str):
                    # per-qchunk exchange: rows [strip u(8), head(2), p(64)]
                    dst = a2a_in[k].rearrange("(u h p) c -> h p u c",
                                              u=8, h=2)
                    nc.sync.dma_start(
                        dst[h], on[:].rearrange("p (u c) -> p u c", u=8))
                else:
                    # rows of a2a_in[k]: [qloc(2), strip i(4), head(2), p(64)]
                    dst = a2a_in[k].rearrange("(q i h p) c -> q h p i c",
                                              q=2, i=4, h=2)
                    nc.sync.dma_start(dst[qloc, h],
                                      on[:].rearrange("p (i c) -> p i c", i=4))

            def send_k(k):
                """Fire one exchange collective; prefetch proj lhs."""
                w = a2a_cols[k]
                nc.gpsimd.collective_compute(
                    "AllToAll", mybir.AluOpType.bypass,
                    replica_groups=[list(range(NCORES))],
                    ins=[a2a_in[k][:]], outs=[a2a_out[k][:]])
                lhs = lpool.tile([128, KT, w], BF16, tag=f"lhs{w}",
                                 name=f"lhs_{k}")
                nc.sync.dma_start(
                    lhs[:], a2a_out[k].rearrange("(j p) c -> p j c", p=128))
                lhs_store[k] = lhs

            # ---- filler schedule: pair-index (2 slots) -> closures ----
            sched = {}

            def at(pair, fn):
                sched.setdefault(pair, []).append(fn)

            for ci, ch in enumerate((1, 2, 3)):          # rest of batch-0 QKV
                for ui, fn in enumerate(chunk_units(ch)):
                    at(ci * 2 + min(ui // 3, 2), fn)
            b1_units = [fn for ch in (4, 5, 6, 7) for fn in chunk_units(ch)]
            for i, fn in enumerate(b1_units):             # batch-1 QKV
                at(8 + (i * 22) // len(b1_units), fn)
            for k, p0 in ((0, 38), (1, 52)):              # out-proj fillers
                at(p0, proj_unit(k, 0))
                at(p0 + 1, proj_unit(k, 1))

            # ---- head: batch-0 chunk 0 QKV ----
            for fn in chunk_units(0):
                fn()

            # ---- 64 slot-pairs (2 key-tiles each): S,S / exp,exp /
            # fillers / AV,AV — fillers absorb the exp latency, and S
            # 64-row-mode matmuls bunch to halve PE mode-switch drains ----
            o_ps = {}
            for pr in range(64):
                g0 = 2 * pr
                bat, qc_l = g0 // 64, (g0 % 64) // 16
                qc = bat * 4 + qc_l
                k = 2 * bat + qc_l // 2
                p_ts = []
                for g in (g0, g0 + 1):
                    mt = g % 16
                    s_t = pspool.tile([128, 2, QC], F32, tag="s", bufs=2,
                                      name=f"s_{g}")
                    for h in range(2):
                        nc.tensor.matmul(
                            s_t[:, h, :],
                            kT_sb[h * 64:(h + 1) * 64,
                                  bat * N + mt * 128:bat * N + (mt + 1) * 128],
                            qT_sb[h * 64:(h + 1) * 64, qc * QC:(qc + 1) * QC],
                            start=True, stop=True)
                    p_t = ppool.tile([128, 2, QC], BF16, tag="p",
                                     name=f"p_{g}")
                    nc.scalar.activation(p_t[:], s_t[:], EXP, scale=SCALE)
                    p_ts.append(p_t)
                for fn in sched.get(pr, ()):
                    fn()
                for gi, g in enumerate((g0, g0 + 1)):
                    mt = g % 16
                    for h in range(2):
                        if mt == 0:
                            o_ps[h] = pspool.tile([65, QC], F32, tag=f"o{h}",
                                                  bufs=1, name=f"o_ps{h}_{qc}")
                        nc.tensor.matmul(
                            o_ps[h][:],
                            v_sb[:, bat * NMT + mt, h * 65:(h + 1) * 65],
                            p_ts[gi][:, h, :],
                            start=(mt == 0), stop=(mt == NMT - 1))
                        if mt == NMT - 1:
                            unit_norm(k, qc_l % 2, h, o_ps[h], fast=(qc == 7))
                if g0 + 1 == 31:
                    send_k(0)
                elif g0 + 1 == 63:
                    send_k(1)
                elif g0 + 1 == 95:
                    send_k(2)

            # ---- tail ----
            send_k(3)
            # proj for earlier collectives here: their lhs waits overlap
            # the last collective's drain instead of stalling attention
            proj_unit(2, 0)()
            proj_unit(2, 1)()
            proj_unit(3, 0)()
            proj_unit(3, 1)()

    nc.compile()
    return nc


def kernel(x, w_qkv, w_proj, b_proj):
    global _NC, LAST_EXEC_NS
    if _NC is None:
        _NC = _build()
    x = np.asarray(x, dtype=np.float32)
    w_qkv = np.asarray(w_qkv, dtype=np.float32)
    w_proj = np.asarray(w_proj, dtype=np.float32)
    b_proj = np.asarray(b_proj, dtype=np.float32)

    import ml_dtypes
    xT = np.ascontiguousarray(x.reshape(NT, C).T).astype(ml_dtypes.bfloat16)
    wpT = np.ascontiguousarray(w_proj.T).astype(ml_dtypes.bfloat16)
    bias = np.ascontiguousarray(b_proj.reshape(1, C))
    idn = np.eye(128, dtype=ml_dtypes.bfloat16)
    in_maps = []
    for c in range(NCORES):
        blk = slice(128 * c, 128 * (c + 1))
        wT = np.ascontiguousarray(
            np.concatenate([w_qkv[0:C][blk], w_qkv[C:2 * C][blk],
                            w_qkv[2 * C:3 * C][blk]], axis=0).T).astype(
                ml_dtypes.bfloat16)
        in_maps.append({"xT": xT, "wT": wT, "wpT": wpT, "bias": bias,
                        "idn": idn})

    if TRACE:
        _install_ntff_hook()
    res = run_bass_kernel_spmd(_NC, in_maps, core_ids=list(range(NCORES)),
                               trace=TRACE)
    LAST_EXEC_NS = res.exec_time_ns
    out = np.empty((B, N, C), dtype=np.float32)
    for c in range(NCORES):
        o = res.results[c]["out"]
        for b in range(B):
            for hb in range(2):
                k = 2 * b + hb
                out[b, 1024 * hb + 128 * c:1024 * hb + 128 * (c + 1), :] = \
                    o[k * 128:(k + 1) * 128, :]
    return np.ascontiguousarray(out)


# revision 43
# speedup vs baseline: 1.1072x; 1.1072x over previous
"""Distributed multi-head attention for 8 TRN2 NeuronCores.

Problem: x[2,2048,1024] -> QKV proj (w_qkv[3072,1024]) -> 16-head SDPA ->
out proj (w_proj[1024,1024] + b_proj) -> [2,2048,1024].

Sharding: 2 heads per core (head-parallel over 8 cores; both batches on
every core). The kernel is one long software pipeline balancing the two
bottleneck engines: scalar-engine EXP (16.8M softmax elements/core ~=
139 us) and the PE (~185 us of matmuls at the observed ~2.0 GHz clock):

  head:  progressive (wT, x) kt-slice DMAs, then QKV for batch-0 chunk 0.
  body:  64 slot-pairs (2 batches x 4 qchunks x 8 key-tile pairs). Each
         pair emits S^T x4 (row-tiled K=64 matmuls, bunched to halve PE
         tile-mode-switch drains) -> EXP x2 -> fillers -> AV x4, where AV
         accumulates [V_h|1]^T P_h^T in PSUM (row 64 = softmax
         denominator).  Fillers occupy the PE during the EXP dependency
         window: remaining batch-0 QKV chunks (pairs 0-5), batch-1 QKV
         (8-29), out-projection for completed collectives (38-39, 52-53).
  norm:  per (qchunk, head): denominator copy, reciprocal_approx_fast,
         gpsimd partition-broadcast, multiply; straight out of PSUM for
         the final qchunk to shorten the chain gating the last exchange.
  A2A:   4 AllToAll collectives (one per half-batch, 0.25MB/core), fired
         as each half-batch is normalized; ~25us each on the Comms engine
         (latency-bound, count matters more than bytes), all but the last
         hidden under later attention.
  proj:  out rows = attnT.T @ w_proj.T + b_proj per 128-token strip;
         strips 0/1 run as fillers, strips 2/3 in the tail where their
         lhs wait overlaps the final collective.

Token ownership: core c owns tokens [1024*hb + 128*c, +128) of each batch
half hb; out_ext row block k=2*b+hb holds that strip. Host reassembles.

Measured: ~278-288us HW exec (baseline 312-340us), rel err 5.4e-3.
"""
import sys, os, types
import numpy as np

if "/opt/trn_rl_repo" not in sys.path and os.path.isdir("/opt/trn_rl_repo"):
    sys.path.append("/opt/trn_rl_repo")

import concourse.bass as bass
import concourse.mybir as mybir
import concourse.tile as tile
from concourse import bacc
from concourse.bass_utils import run_bass_kernel_spmd

F32 = mybir.dt.float32
BF16 = mybir.dt.bfloat16
EXP = mybir.ActivationFunctionType.Exp
MULT = mybir.AluOpType.mult
ADD = mybir.AluOpType.add

NCORES = 8
B, N, C, H, D = 2, 2048, 1024, 16, 64
NT = B * N          # 4096 flat tokens
KT = C // 128       # 8 contraction tiles of 128
QC = 512            # query-chunk width (one PSUM bank of f32)
NMT = N // 128      # 16 key tiles per batch
SCALE = 1.0 / 8.0   # 1/sqrt(D)
NCH = NT // QC      # 8 512-token QKV chunks (4 per batch)

TRACE = False       # test harness sets True to capture exec_time_ns
LAST_EXEC_NS = None

_NC = None


def _install_ntff_hook():
    if "antenv.axon_hooks" in sys.modules:
        return
    try:
        import antenv
        from trn_agent_boot.trn_boot import _ntff_profile_via_ctypes
        mod = types.ModuleType("antenv.axon_hooks")
        _hook = [None]
        mod.set_axon_ntff_profile_hook = lambda h: _hook.__setitem__(0, h)
        mod.get_axon_ntff_profile_hook = lambda: _hook[0]
        sys.modules["antenv.axon_hooks"] = mod
        antenv.axon_hooks = mod
        mod.set_axon_ntff_profile_hook(
            _ntff_profile_via_ctypes("/opt/axon/libaxon_pjrt.so"))
    except Exception:
        pass


def _build():
    nc = bacc.Bacc("TRN2", target_bir_lowering=False, debug=False,
                   num_devices=NCORES)
    xT_ext = nc.dram_tensor("xT", [C, NT], BF16, kind="ExternalInput").ap()
    wT_ext = nc.dram_tensor("wT", [C, 384], BF16, kind="ExternalInput").ap()
    wpT_ext = nc.dram_tensor("wpT", [C, C], BF16, kind="ExternalInput").ap()
    bias_ext = nc.dram_tensor("bias", [1, C], F32, kind="ExternalInput").ap()
    idn_ext = nc.dram_tensor("idn", [128, 128], BF16, kind="ExternalInput").ap()
    out_ext = nc.dram_tensor("out", [NT // NCORES, C], F32,
                             kind="ExternalOutput").ap()
    # one exchange per half-batch (128-token strips/core); each AllToAll
    # costs ~25us on the Comms engine regardless of payload, so fewer,
    # overlapped collectives beat finer splits
    a2a_cols = {0: 128, 1: 128, 2: 128, 3: 128}
    a2a_in = {k: nc.dram_tensor(f"a2a_in{k}", [NCORES * 128, w], BF16).ap()
              for k, w in a2a_cols.items()}
    a2a_out = {k: nc.dram_tensor(f"a2a_out{k}", [NCORES * 128, w], BF16).ap()
               for k, w in a2a_cols.items()}

    xT_v = xT_ext.rearrange("(kt p) n -> p kt n", p=128)
    wT_v = wT_ext.rearrange("(kt p) f -> p kt f", p=128)
    wpT_v = wpT_ext.rearrange("(kt p) f -> p kt f", p=128)

    with tile.TileContext(nc) as tc:
        with (
            tc.tile_pool(name="const", bufs=1) as cpool,
            tc.tile_pool(name="resid", bufs=1) as rpool,
            tc.tile_pool(name="xchunk", bufs=1) as xpool,
            tc.tile_pool(name="vtmp", bufs=2) as vpool,
            tc.tile_pool(name="pexp", bufs=4) as ppool,
            tc.tile_pool(name="stg", bufs=2) as stpool,
            tc.tile_pool(name="denp", bufs=2) as dpool,
            tc.tile_pool(name="rbp", bufs=4) as rbpool,
            tc.tile_pool(name="onrm", bufs=4) as onpool,
            tc.tile_pool(name="plhs", bufs=2) as lpool,
            tc.tile_pool(name="pout", bufs=2) as outpool,
            tc.tile_pool(name="ps", bufs=1, space="PSUM") as pspool,
        ):
            # ---- constants + input DMAs (few large issues; DIRECT2D costs
            # ~0.6us of Sync-queue time per dma_start) ----
            wT_sb = cpool.tile([128, KT, 384], BF16)
            x_t = []
            for i in range(NT // 1024):
                xt = xpool.tile([128, KT, 1024], BF16, tag=f"x{i}",
                                name=f"x_{i}")
                x_t.append(xt)
            # first QKV group consumes (wT, x0) kt-slices in order: land
            # them progressively so the cold-start matmuls stream behind DMA
            for kt0 in range(0, KT, 2):
                nc.sync.dma_start(wT_sb[:, kt0:kt0 + 2, :],
                                  wT_v[:, kt0:kt0 + 2, :])
                nc.sync.dma_start(x_t[0][:, kt0:kt0 + 2, :],
                                  xT_v[:, kt0:kt0 + 2, 0:1024])
            idn = cpool.tile([128, 128], BF16)
            nc.sync.dma_start(idn[:], idn_ext[:])
            bias_sb = cpool.tile([1, C], F32)
            nc.sync.dma_start(bias_sb[:], bias_ext[:])
            bias_bc = cpool.tile([128, C], F32)
            nc.gpsimd.partition_broadcast(bias_bc[:], bias_sb[:])
            for i in range(1, 4):
                nc.sync.dma_start(x_t[i][:],
                                  xT_v[:, :, i * 1024:(i + 1) * 1024])

            qT_sb = rpool.tile([128, NT], BF16)
            kT_sb = rpool.tile([128, NT], BF16)
            v_sb = rpool.tile([128, NT // 128, 130], BF16)
            nc.gpsimd.memset(v_sb[:, :, 64], 1.0)
            nc.gpsimd.memset(v_sb[:, :, 129], 1.0)
            wp_sb = rpool.tile([128, KT, C], BF16)
            nc.sync.dma_start(wp_sb[:], wpT_v[:])

            vt_store = {}
            lhs_store = {}

            # ---- filler closures ----
            def qkv_unit(ch, ft, c0=0, c1=QC):
                """One QKV matmul group: tokens [c0,c1) of 512-token chunk
                ch x one feature block (ft 0=q, 1=k, 2=v) accumulated over
                KT, then evacuated."""
                def emit():
                    ti, hw = ch // 2, ch % 2
                    ncol = ch * QC + c0
                    w = c1 - c0
                    ps = pspool.tile([128, w], F32, tag="s", bufs=2,
                                     name=f"qkvps_{ch}_{ft}_{c0}")
                    for kt in range(KT):
                        nc.tensor.matmul(
                            ps[:],
                            wT_sb[:, kt, ft * 128:(ft + 1) * 128],
                            x_t[ti][:, kt, hw * QC + c0:hw * QC + c1],
                            start=(kt == 0), stop=(kt == KT - 1))
                    if ft == 0:
                        nc.vector.tensor_copy(out=qT_sb[:, ncol:ncol + w],
                                              in_=ps[:])
                    elif ft == 1:
                        nc.vector.tensor_copy(out=kT_sb[:, ncol:ncol + w],
                                              in_=ps[:])
                    else:
                        vt = vpool.tile([128, QC], BF16, tag="vt",
                                        name=f"vt_{ch}")
                        nc.vector.tensor_copy(out=vt[:], in_=ps[:])
                        vt_store[ch] = vt
                return emit

            def tr_unit(ch, t):
                """PE-transpose one 128-token tile of V into natural layout."""
                def emit():
                    mtg = ch * 4 + t
                    trp = pspool.tile([128, 128], BF16, tag="tr", bufs=2,
                                      name=f"tr_{mtg}")
                    nc.tensor.transpose(trp[:], vt_store[ch][:, t * 128:(t + 1) * 128],
                                        idn[:])
                    nc.vector.tensor_copy(out=v_sb[:, mtg, 0:64],
                                          in_=trp[:, 0:64])
                    nc.vector.tensor_copy(out=v_sb[:, mtg, 65:129],
                                          in_=trp[:, 64:128])
                return emit

            out_row = {0: 0, 1: 128, 2: 256, 3: 384}

            def proj_unit(k, half):
                """Out-projection for strip k, 512 output cols."""
                def emit():
                    lhs = lhs_store[k]
                    w = a2a_cols[k]
                    pp = pspool.tile([w, QC], F32, tag="s", bufs=2,
                                     name=f"pp_{k}_{half}")
                    for j in range(KT):
                        nc.tensor.matmul(
                            pp[:], lhs[:, j, :],
                            wp_sb[:, j, half * QC:(half + 1) * QC],
                            start=(j == 0), stop=(j == KT - 1))
                    ot = outpool.tile([w, QC], F32, tag="ot",
                                      name=f"ot_{k}_{half}")
                    nc.vector.tensor_tensor(ot[:], pp[:],
                                            bias_bc[0:w, half * QC:(half + 1) * QC],
                                            ADD)
                    nc.sync.dma_start(
                        out_ext[out_row[k]:out_row[k] + w,
                                half * QC:(half + 1) * QC], ot[:])
                return emit

            def chunk_units(ch):
                # k then q first: the first score matmul of a qchunk only
                # needs kT + qT, so exp starts before v is transposed
                return [qkv_unit(ch, 1), qkv_unit(ch, 0), qkv_unit(ch, 2),
                        tr_unit(ch, 0), tr_unit(ch, 1),
                        tr_unit(ch, 2), tr_unit(ch, 3)]

            def unit_norm(k, qloc, h, o_ps_h, fast=False):
                """Evacuate one finished (qchunk, head) PSUM accumulator,
                normalize by its softmax denominator, stream into a2a_in.
                fast=True (final qchunk) works straight out of PSUM to
                shorten the chain gating the last collective."""
                uid = f"{k}_{qloc}_{h}"
                dn = dpool.tile([1, QC], F32, tag="den", bufs=4,
                                name=f"den_{uid}")
                nc.vector.tensor_copy(out=dn[:], in_=o_ps_h[64:65, :])
                if fast:
                    src = o_ps_h[0:64, :]
                else:
                    st = stpool.tile([64, QC], F32, tag="st", bufs=4,
                                     name=f"st_{uid}")
                    nc.vector.tensor_copy(out=st[:], in_=o_ps_h[0:64, :])
                    src = st[:]
                rcp = dpool.tile([1, QC], F32, tag="rcp", bufs=4,
                                 name=f"rcp_{uid}")
                nc.vector.reciprocal_approx_fast(rcp[:], dn[:])
                rb = rbpool.tile([64, QC], F32, tag="rb", name=f"rb_{uid}")
                nc.gpsimd.partition_broadcast(rb[:], rcp[:])
                on = onpool.tile([64, QC], BF16, tag="on", name=f"on_{uid}")
                nc.vector.tensor_tensor(on[:], src, rb[:], MULT)
                if isinstance(k, str):
                    # per-qchunk exchange: rows [strip u(8), head(2), p(64)]
                    dst = a2a_in[k].rearrange("(u h p) c -> h p u c",
                                              u=8, h=2)
                    nc.sync.dma_start(
                        dst[h], on[:].rearrange("p (u c) -> p u c", u=8))
                else:
                    # rows of a2a_in[k]: [qloc(2), strip i(4), head(2), p(64)]
                    dst = a2a_in[k].rearrange("(q i h p) c -> q h p i c",
                                              q=2, i=4, h=2)
                    nc.sync.dma_start(dst[qloc, h],
                                      on[:].rearrange("p (i c) -> p i c", i=4))

            def send_k(k):
                """Fire one exchange collective; prefetch proj lhs."""
                w = a2a_cols[k]
                nc.gpsimd.collective_compute(
                    "AllToAll", mybir.AluOpType.bypass,
                    replica_groups=[list(range(NCORES))],
                    ins=[a2a_in[k][:]], outs=[a2a_out[k][:]])
                lhs = lpool.tile([128, KT, w], BF16, tag=f"lhs{w}",
                                 name=f"lhs_{k}")
                nc.sync.dma_start(
                    lhs[:], a2a_out[k].rearrange("(j p) c -> p j c", p=128))
                lhs_store[k] = lhs

            # ---- filler schedule: pair-index (2 slots) -> closures ----
            sched = {}

            def at(pair, fn):
                sched.setdefault(pair, []).append(fn)

            for ci, ch in enumerate((1, 2, 3)):          # rest of batch-0 QKV
                for ui, fn in enumerate(chunk_units(ch)):
                    at(ci * 2 + min(ui // 3, 2), fn)
            b1_units = [fn for ch in (4, 5, 6, 7) for fn in chunk_units(ch)]
            for i, fn in enumerate(b1_units):             # batch-1 QKV
                at(8 + (i * 22) // len(b1_units), fn)
            for k, p0 in ((0, 38), (1, 52)):              # out-proj fillers
                at(p0, proj_unit(k, 0))
                at(p0 + 1, proj_unit(k, 1))

            # ---- head: dummy idn matmuls during the x-DMA wait warm the
            # HAM clock-gate (~3.4us of PE activity) so chunk-0 QKV runs
            # at 2.4GHz instead of the cold 1.2GHz (idn lands ~8us, x ~13us)
            for i in range(40):
                hw_ps = pspool.tile([128, 128], F32, tag="tr", bufs=2,
                                    name=f"hamw_{i}")
                nc.tensor.matmul(hw_ps[:], idn[:], idn[:],
                                 start=True, stop=True)
            # ---- batch-0 chunk 0 QKV ----
            for fn in chunk_units(0):
                fn()

            # ---- 64 slot-pairs (2 key-tiles each): S,S / exp,exp /
            # fillers / AV,AV — fillers absorb the exp latency, and S
            # 64-row-mode matmuls bunch to halve PE mode-switch drains ----
            o_ps = {}
            for pr in range(64):
                g0 = 2 * pr
                bat, qc_l = g0 // 64, (g0 % 64) // 16
                qc = bat * 4 + qc_l
                k = 2 * bat + qc_l // 2
                p_ts = []
                for g in (g0, g0 + 1):
                    mt = g % 16
                    s_t = pspool.tile([128, 2, QC], F32, tag="s", bufs=2,
                                      name=f"s_{g}")
                    for h in range(2):
                        nc.tensor.matmul(
                            s_t[:, h, :],
                            kT_sb[h * 64:(h + 1) * 64,
                                  bat * N + mt * 128:bat * N + (mt + 1) * 128],
                            qT_sb[h * 64:(h + 1) * 64, qc * QC:(qc + 1) * QC],
                            start=True, stop=True)
                    p_t = ppool.tile([128, 2, QC], BF16, tag="p",
                                     name=f"p_{g}")
                    nc.scalar.activation(p_t[:], s_t[:], EXP, scale=SCALE)
                    p_ts.append(p_t)
                for fn in sched.get(pr, ()):
                    fn()
                for gi, g in enumerate((g0, g0 + 1)):
                    mt = g % 16
                    for h in range(2):
                        if mt == 0:
                            o_ps[h] = pspool.tile([65, QC], F32, tag=f"o{h}",
                                                  bufs=1, name=f"o_ps{h}_{qc}")
                        nc.tensor.matmul(
                            o_ps[h][:],
                            v_sb[:, bat * NMT + mt, h * 65:(h + 1) * 65],
                            p_ts[gi][:, h, :],
                            start=(mt == 0), stop=(mt == NMT - 1))
                        if mt == NMT - 1:
                            unit_norm(k, qc_l % 2, h, o_ps[h], fast=(qc == 7))
                if g0 + 1 == 31:
                    send_k(0)
                elif g0 + 1 == 63:
                    send_k(1)
                elif g0 + 1 == 95:
                    send_k(2)

            # ---- tail ----
            send_k(3)
            # proj for earlier collectives here: their lhs waits overlap
            # the last collective's drain instead of stalling attention
            proj_unit(2, 0)()
            proj_unit(2, 1)()
            proj_unit(3, 0)()
            proj_unit(3, 1)()

    nc.compile()
    return nc


def kernel(x, w_qkv, w_proj, b_proj):
    global _NC, LAST_EXEC_NS
    if _NC is None:
        _NC = _build()
    x = np.asarray(x, dtype=np.float32)
    w_qkv = np.asarray(w_qkv, dtype=np.float32)
    w_proj = np.asarray(w_proj, dtype=np.float32)
    b_proj = np.asarray(b_proj, dtype=np.float32)

    import ml_dtypes
    xT = np.ascontiguousarray(x.reshape(NT, C).T).astype(ml_dtypes.bfloat16)
    wpT = np.ascontiguousarray(w_proj.T).astype(ml_dtypes.bfloat16)
    bias = np.ascontiguousarray(b_proj.reshape(1, C))
    idn = np.eye(128, dtype=ml_dtypes.bfloat16)
    in_maps = []
    for c in range(NCORES):
        blk = slice(128 * c, 128 * (c + 1))
        wT = np.ascontiguousarray(
            np.concatenate([w_qkv[0:C][blk], w_qkv[C:2 * C][blk],
                            w_qkv[2 * C:3 * C][blk]], axis=0).T).astype(
                ml_dtypes.bfloat16)
        in_maps.append({"xT": xT, "wT": wT, "wpT": wpT, "bias": bias,
                        "idn": idn})

    if TRACE:
        _install_ntff_hook()
    res = run_bass_kernel_spmd(_NC, in_maps, core_ids=list(range(NCORES)),
                               trace=TRACE)
    LAST_EXEC_NS = res.exec_time_ns
    out = np.empty((B, N, C), dtype=np.float32)
    for c in range(NCORES):
        o = res.results[c]["out"]
        for b in range(B):
            for hb in range(2):
                k = 2 * b + hb
                out[b, 1024 * hb + 128 * c:1024 * hb + 128 * (c + 1), :] = \
                    o[k * 128:(k + 1) * 128, :]
    return np.ascontiguousarray(out)
